# revision 57
# baseline (speedup 1.0000x reference)
"""NearbyAttention Trainium2 kernel.

Full-input contract: kernel(**inputs) takes the unsharded numpy inputs of
nn_NearbyAttention (q,k,v: [16,1025,1024] f32; Wq/Wk/Wv/Wo: [1024,1024] f32;
bo: [1024] f32) and returns the full [16,1025,1024] f32 output.

Strategy: 8-way data parallel over the batch dim (2 batches per NeuronCore),
weights replicated, no collectives. Host pre-transposes activations to
[dim, tokens] and casts to bf16. On device, per batch:
  - v projection ktile-outer in token-pair groups so the PE starts on the
    first DMA'd contraction chunks; evacuation on the scalar engine
  - sparse "nearby" attention exploiting the 5x5 locality mask:
    key chunks of 128 (4 patch rows) x uniform 256-query windows in S^T
    layout; exp on the scalar engine with the 1/sqrt(d) scale folded into
    the activation's affine; multiplicative per-chunk-pair masks (4 shared
    patterns); PV accumulation runs two chunk pairs behind QK so the
    exp->mask chain never stalls the in-order PE queue; a ones-column in
    the value panel yields the softmax denominator for free
  - BOS-as-key scores packed 4-rows-to-a-PSUM-bank via matmul col tiling
    (one exp covers a whole head pair); their rank-1 PV outer products are
    the LAST accumulation into each pv bank, off the critical path
  - BOS-as-query output for all 16 heads packed into one PSUM bank
    (head -> column), with incremental per-head-pair reciprocals
  - head-pair loop software-pipelined (projections of hp+1 between
    attention and normalize of hp); odd-head attnout rows reach their
    partitions via small SBUF->SBUF shift DMAs on the gpsimd queue
  - output projection split into open (head pairs 0-6) / close (7 +
    evacuation) per token chunk, two chunks in flight, so the contraction
    never waits on the last head pair's normalize; next batch's input DMAs
    are emitted ahead of this batch's output DMAs in the sync-queue FIFO
"""

import numpy as np
import ml_dtypes

import concourse.bass as bass
import concourse.mybir as mybir
import concourse.tile as tile
from concourse import bacc
from concourse.bass_utils import run_bass_kernel_spmd

BF16 = mybir.dt.bfloat16
F32 = mybir.dt.float32
AF = mybir.ActivationFunctionType
ALU = mybir.AluOpType

B = 16              # full batch
BPC = 2             # batches per core
NCORES = 8
NT = 1025           # tokens (BOS + 32*32 grid)
G = 1024            # grid tokens
DIM = 1024
HEADS = 16
DH = 64
INNER = HEADS * DH  # 1024
P = 128
SCALE = DH ** -0.5  # 0.125
NEG = -30.0         # mask bias for the handful of memset'd cells

KC = 8              # key chunks of 128 grid tokens (4 patch rows each)


def _qwin(kc: int) -> int:
    """Grid-query window start for key chunk kc (width always 256)."""
    return min(max(128 * kc - 64, 0), G - 256)


def _masks() -> np.ndarray:
    """[128, 4*256] 0/1 bf16 masks in S^T orientation: patterns for the
    first chunk, interior (x2 so any chunk pair is one contiguous 512
    slice), and last chunk. Chunk pair kcp uses cols 256*kcp .. +512."""
    out = np.zeros((128, 4 * 256), dtype=ml_dtypes.bfloat16)
    kr_off = np.arange(128)[:, None] // 32   # key row within chunk
    kcol = np.arange(128)[:, None] % 32
    for mi, kc in ((0, 0), (1, 1), (2, 1), (3, KC - 1)):
        g0 = _qwin(kc)
        kr = 4 * kc + kr_off                 # absolute key row
        q = g0 + np.arange(256)[None, :]
        qr = q // 32
        qcol = q % 32
        m = (np.abs(kr - qr) <= 2) & (np.abs(kcol - qcol) <= 2)
        out[:, 256 * mi: 256 * (mi + 1)] = m.astype(ml_dtypes.bfloat16)
    return out


def build_nc():
    nc = bacc.Bacc("TRN2", target_bir_lowering=False, debug=False,
                   num_devices=NCORES)

    qt = nc.dram_tensor("qt", [BPC, DIM, NT], BF16, kind="ExternalInput")
    kt = nc.dram_tensor("kt", [BPC, DIM, NT], BF16, kind="ExternalInput")
    vt = nc.dram_tensor("vt", [BPC, DIM, NT], BF16, kind="ExternalInput")
    wq = nc.dram_tensor("wq", [DIM, INNER], BF16, kind="ExternalInput")
    wk = nc.dram_tensor("wk", [DIM, INNER], BF16, kind="ExternalInput")
    wv = nc.dram_tensor("wv", [DIM, INNER], BF16, kind="ExternalInput")
    wo = nc.dram_tensor("wo", [INNER, DIM], BF16, kind="ExternalInput")
    out = nc.dram_tensor("out", [BPC, NT, DIM], F32, kind="ExternalOutput")

    mask_dram = nc.inline_tensor(_masks(), name="maskall")

    HH = (slice(0, 64), slice(64, 128))

    with tile.TileContext(nc) as tc:
        with (
            tc.tile_pool(name="singles", bufs=1) as singles,
            tc.tile_pool(name="perbatch", bufs=1) as perbatch,
            tc.tile_pool(name="vpool", bufs=2) as vpool,
            tc.tile_pool(name="hppool", bufs=2) as hppool,
            tc.tile_pool(name="ppool", bufs=4) as ppool,
            tc.tile_pool(name="small", bufs=2) as small,
            tc.tile_pool(name="bcast", bufs=2) as bcast,
            tc.tile_pool(name="psA", bufs=4, space="PSUM") as psA,
            tc.tile_pool(name="psPV", bufs=4, space="PSUM") as psPV,
        ):
            # ---- persistent weights/constants ----
            wq_sb = singles.tile([P, 8, INNER], BF16, tag="wq")
            wk_sb = singles.tile([P, 8, INNER], BF16, tag="wk")
            wv_sb = singles.tile([P, 8, INNER], BF16, tag="wv")
            wo_sb = singles.tile([P, 8, DIM], BF16, tag="wo")
            maskall_sb = singles.tile([P, 4 * 256], BF16, tag="maskall")
            ones_sb = singles.tile([P, 1], BF16, tag="ones")

            wv_r = wv.ap().rearrange("(c p) n -> p c n", p=P)
            wq_r = wq.ap().rearrange("(c p) n -> p c n", p=P)
            wk_r = wk.ap().rearrange("(c p) n -> p c n", p=P)
            wo_r = wo.ap().rearrange("(c p) n -> p c n", p=P)

            nc.vector.memset(ones_sb[:], 1.0)

            def load_qk_weights():
                for kt_ in range(8):
                    nc.sync.dma_start(wq_sb[:, kt_], wq_r[:, kt_])
                    nc.sync.dma_start(wk_sb[:, kt_], wk_r[:, kt_])
                nc.sync.dma_start(maskall_sb[:], mask_dram[:])
                nc.sync.dma_start(wo_sb[:], wo_r)

            def load_batch(b, qT, kT, vT):
                vt_r = vt[b].rearrange("(c p) n -> p c n", p=P)
                qt_r = qt[b].rearrange("(c p) n -> p c n", p=P)
                kt_r = kt[b].rearrange("(c p) n -> p c n", p=P)
                for kt_ in range(8):
                    if b == 0:
                        nc.sync.dma_start(wv_sb[:, kt_], wv_r[:, kt_])
                    nc.sync.dma_start(vT[:, kt_], vt_r[:, kt_])
                for kt_ in range(8):
                    nc.sync.dma_start(qT[:, kt_], qt_r[:, kt_])
                    nc.sync.dma_start(kT[:, kt_], kt_r[:, kt_])
                if b == 0:
                    load_qk_weights()

            load_maps = {}
            for b in range(BPC):
                # ---- transposed activations (loads for batch b+1 are
                # emitted before batch b's output projection, ahead of the
                # output DMAs in the sync-queue FIFO) ----
                if b == 0:
                    qT = perbatch.tile([P, 8, NT], BF16, tag="qT")
                    kT = perbatch.tile([P, 8, NT], BF16, tag="kT")
                    vT = vpool.tile([P, 8, NT], BF16, tag="vT")
                    load_batch(0, qT, kT, vT)
                else:
                    qT, kT, vT = load_maps.pop(b)

                # ---- v projection into head panels ----
                # vh_panel[g, kc, h, 0:64] = (v @ Wv)[token g+1, h*64:...]
                # col 64 = ones (softmax denominator trick)
                vh_panel = perbatch.tile([P, KC, HEADS, DH + 1], BF16, tag="vhp")
                vbos_panel = perbatch.tile([1, HEADS, DH + 1], BF16, tag="vbos")
                vbos4 = perbatch.tile([97, HEADS, DH + 1], BF16, tag="vbos4")
                nc.vector.memset(vh_panel[:, :, :, DH], 1.0)
                nc.vector.memset(vbos_panel[:, :, DH], 1.0)

                # ktile-outer accumulation in mt-pair groups so compute can
                # start as soon as the first ktile chunks land
                for g in range(4):
                    accs = [[psA.tile([P, 512], F32, tag="ps",
                                      name=f"acc{_m}_{_h}")
                             for _h in range(2)] for _m in range(2)]
                    for ktile in range(8):
                        for mi in range(2):
                            mt = 2 * g + mi
                            for half in range(2):
                                nc.tensor.matmul(
                                    accs[mi][half][:],
                                    vT[:, ktile, 1 + 128 * mt: 1 + 128 * (mt + 1)],
                                    wv_sb[:, ktile, 512 * half: 512 * (half + 1)],
                                    start=(ktile == 0), stop=(ktile == 7),
                                )
                    for mi in range(2):
                        mt = 2 * g + mi
                        for half in range(2):
                            nc.scalar.copy(
                                vh_panel[:, mt, 8 * half: 8 * (half + 1), 0:DH],
                                accs[mi][half].rearrange("p (h d) -> p h d", d=DH),
                            )
                # BOS token of v
                for half in range(2):
                    acc = psA.tile([P, 512], F32, tag="ps")
                    for ktile in range(8):
                        nc.tensor.matmul(
                            acc[0:1, :], vT[:, ktile, 0:1],
                            wv_sb[:, ktile, 512 * half: 512 * (half + 1)],
                            start=(ktile == 0), stop=(ktile == 7),
                        )
                    nc.scalar.copy(
                        vbos_panel[:, 8 * half: 8 * (half + 1), 0:DH],
                        acc[0:1].rearrange("p (h d) -> p h d", d=DH),
                    )
                # replicate vbos to partitions 0/32/64/96 for the packed
                # BOS-as-key outer products
                nc.gpsimd.partition_broadcast(
                    vbos4.rearrange("p h d -> p (h d)"),
                    vbos_panel.rearrange("p h d -> p (h d)"))

                attnout = perbatch.tile([P, 8, NT], BF16, tag="attnout")
                zq_sb = small.tile([1, HEADS], F32, tag="zq")
                pq_all = perbatch.tile([P, HEADS, 9], BF16, tag="pqall")
                rzq_all = perbatch.tile([1, HEADS], F32, tag="rzq")
                rzqb_all = perbatch.tile([P, HEADS], F32, tag="rzqb")

                def project_chunks(hp):
                    """q/k head-pair projection emitters for inner chunk hp.
                    Returns (qhT, khT, [chunk emitters]); each emitter does
                    one 8-matmul PSUM group plus its evacuation. Ordered so
                    the windows attention(hp) touches first land first."""
                    qhT = hppool.tile([P, NT], BF16, tag="qhT")
                    khT = hppool.tile([P, NT], BF16, tag="khT")
                    plan = [
                        (khT, wk_sb, kT, 0, 512, "dve"),
                        (qhT, wq_sb, qT, 0, 512, "act"),
                        (khT, wk_sb, kT, 512, 512, "act"),
                        (qhT, wq_sb, qT, 512, 512, "dve"),
                        (khT, wk_sb, kT, 1024, 1, "dve"),
                        (qhT, wq_sb, qT, 1024, 1, "act"),
                    ]

                    def mk(dst, w_sb, src, nt0, ntw, eng):
                        def emit():
                            acc = psA.tile([P, 512], F32, tag="ps")
                            for ktile in range(8):
                                nc.tensor.matmul(
                                    acc[:, 0:ntw],
                                    w_sb[:, ktile, 128 * hp: 128 * (hp + 1)],
                                    src[:, ktile, nt0: nt0 + ntw],
                                    start=(ktile == 0), stop=(ktile == 7),
                                )
                            if eng == "act":
                                nc.scalar.copy(dst[:, nt0: nt0 + ntw],
                                               acc[:, 0:ntw])
                            else:
                                nc.vector.tensor_copy(dst[:, nt0: nt0 + ntw],
                                                      acc[:, 0:ntw])
                        return emit

                    return qhT, khT, [mk(*args) for args in plan]

                def attention(hp, qhT, khT, mid_fill):
                    """Returns pv2 (PSUM tiles, stopped) for normalize().

                    Emits the next head pair's projection chunks between this
                    pair's QK->exp->mask->PV stages so the (in-order) PE
                    queue always has ready matmuls while ACT/DVE work on the
                    softmax. The BOS-as-key outer product is the LAST
                    accumulation into each pv bank so its score/exp chain is
                    off the critical path."""
                    pv2 = [[psPV.tile([DH + 1, 512], F32, tag="pv",
                                      name=f"pv{_i}_{_j}") for _j in range(2)]
                           for _i in range(2)]
                    started = [[False, False], [False, False]]

                    def emit_pv(kcp, p2):
                        for hh in range(2):
                            h = 2 * hp + hh
                            for j, kc in enumerate((2 * kcp, 2 * kcp + 1)):
                                g0 = _qwin(kc)
                                if g0 < 512 and g0 + 256 > 512:
                                    pieces = [(g0, 512 - g0),
                                              (512, g0 + 256 - 512)]
                                else:
                                    pieces = [(g0, 256)]
                                off = 0
                                for pg0, pw in pieces:
                                    half = 1 if pg0 >= 512 else 0
                                    nc.tensor.matmul(
                                        pv2[hh][half][:, pg0 - 512 * half:
                                                       pg0 - 512 * half + pw],
                                        vh_panel[:, kc, h, :],
                                        p2[hh][:, 256 * j + off:
                                                256 * j + off + pw],
                                        start=not started[hh][half],
                                        stop=False, skip_group_check=True,
                                    )
                                    started[hh][half] = True
                                    off += pw

                    prev = []
                    for kcp in range(4):  # pairs of key chunks
                        kca, kcb = 2 * kcp, 2 * kcp + 1
                        s2 = [psA.tile([P, 512], F32, tag="ps", name=f"s2_{_i}")
                              for _i in range(2)]
                        p2 = [ppool.tile([P, 512], BF16, tag="p", bufs=6,
                                         name=f"p2_{_i}")
                              for _i in range(2)]
                        for j, kc in enumerate((kca, kcb)):
                            g0 = _qwin(kc)
                            for hh in range(2):
                                nc.tensor.matmul(
                                    s2[hh][:, 256 * j: 256 * (j + 1)],
                                    khT[HH[hh], 1 + 128 * kc: 1 + 128 * (kc + 1)],
                                    qhT[HH[hh], 1 + g0: 1 + g0 + 256],
                                    start=True, stop=True,
                                )
                        m0 = 256 * {0: 0, 1: 1, 2: 1, 3: 2}[kcp]
                        for hh in range(2):
                            nc.scalar.activation(p2[hh][:], s2[hh][:], AF.Exp,
                                                 scale=SCALE)
                            nc.vector.tensor_tensor(
                                p2[hh][:], p2[hh][:],
                                maskall_sb[:, m0: m0 + 512], ALU.mult)
                        # PV runs TWO chunk pairs behind QK: the exp->mask
                        # chain (~2us) completes while the in-order PE queue
                        # works through the two newer QK groups
                        prev.append((kcp, p2))
                        if len(prev) > 2:
                            emit_pv(*prev.pop(0))

                        if kcp == 0:
                            # ---- BOS-as-key scores, 4 rows in one bank ----
                            sbq = psA.tile([P, 512], F32, tag="ps")
                            for hh in range(2):
                                for half in range(2):
                                    r = 32 * (2 * hh + half)
                                    nc.tensor.matmul(
                                        sbq[r: r + 1, :], khT[HH[hh], 0:1],
                                        qhT[HH[hh],
                                            1 + 512 * half: 1 + 512 * (half + 1)],
                                        start=True, stop=True,
                                        skip_group_check=True,
                                        tile_position=(64 * hh, r),
                                    )
                            pbos = ppool.tile([P, 512], BF16, tag="pbos",
                                              bufs=2)
                            nc.scalar.activation(pbos[0:97, :], sbq[0:97, :],
                                                 AF.Exp, scale=SCALE)
                        if kcp == 1:
                            # ---- BOS-as-query scores over everything ----
                            sq = psA.tile([P, 512], F32, tag="ps")
                            for hh in range(2):
                                nc.vector.memset(sq[:, 9 * hh: 9 * hh + 1],
                                                 NEG / SCALE)
                            for hh in range(2):
                                nc.tensor.matmul(sq[0:1, 9 * hh: 9 * hh + 1],
                                                 khT[HH[hh], 0:1],
                                                 qhT[HH[hh], 0:1],
                                                 start=True, stop=True,
                                                 skip_group_check=True)
                                for c in range(8):
                                    nc.tensor.matmul(
                                        sq[:, 9 * hh + 1 + c: 9 * hh + 2 + c],
                                        khT[HH[hh], 1 + 128 * c: 1 + 128 * (c + 1)],
                                        qhT[HH[hh], 0:1], start=True, stop=True,
                                        skip_group_check=True,
                                    )
                            nc.scalar.activation(
                                pq_all[:, 2 * hp: 2 * hp + 2, :], sq[:, 0:18],
                                AF.Exp, scale=SCALE)
                            zrow = psA.tile([P, 512], F32, tag="ps")
                            nc.tensor.matmul(zrow[0:1, 0:18], ones_sb[:],
                                             pq_all[:, 2 * hp: 2 * hp + 2, :],
                                             start=True, stop=True)
                            for hh in range(2):
                                h = 2 * hp + hh
                                nc.vector.tensor_reduce(
                                    zq_sb[:, h: h + 1],
                                    zrow[0:1, 9 * hh: 9 * hh + 9],
                                    axis=mybir.AxisListType.X, op=ALU.add)
                            # incremental BOS-query reciprocal + broadcast so
                            # the end-of-batch epilogue has no serial chain
                            nc.vector.reciprocal_approx_fast(
                                rzq_all[:, 2 * hp: 2 * hp + 2],
                                zq_sb[:, 2 * hp: 2 * hp + 2])
                            nc.gpsimd.partition_broadcast(
                                rzqb_all[:, 2 * hp: 2 * hp + 2],
                                rzq_all[0:1, 2 * hp: 2 * hp + 2])

                        if kcp == 1 and mid_fill is not None:
                            mid_fill()

                    for pr in prev:
                        emit_pv(*pr)
                    # BOS-as-key outer products close each accumulation group
                    for hh in range(2):
                        h = 2 * hp + hh
                        for j in range(2):
                            r = 32 * (2 * hh + j)
                            nc.tensor.matmul(
                                pv2[hh][j][:], vbos4[r: r + 1, h, :],
                                pbos[r: r + 1, 0:512],
                                start=False, stop=True, skip_group_check=True,
                                tile_position=(r, 0))
                    return pv2

                def normalize_hh(hp, pv2, hh):
                    rzb = bcast.tile([64, G], F32, tag="rzb")
                    rz = small.tile([1, G], F32, tag="rz")
                    for j in range(2):
                        if hh == 0:
                            nc.scalar.copy(rz[:, 512 * j: 512 * (j + 1)],
                                           pv2[hh][j][DH: DH + 1, :])
                        else:
                            nc.vector.tensor_copy(rz[:, 512 * j: 512 * (j + 1)],
                                                  pv2[hh][j][DH: DH + 1, :])
                    nc.vector.reciprocal_approx_fast(rz[:], rz[:])
                    nc.gpsimd.partition_broadcast(rzb[:], rz[:])
                    if hh == 0:
                        for j in range(2):
                            nc.vector.tensor_tensor(
                                attnout[HH[0], hp, 1 + 512 * j:
                                        1 + 512 * (j + 1)],
                                pv2[0][j][0:DH, :],
                                rzb[:, 512 * j: 512 * (j + 1)], ALU.mult)
                    else:
                        tmp = bcast.tile([64, G], BF16, tag="tmp1")
                        for j in range(2):
                            nc.vector.tensor_tensor(
                                tmp[:, 512 * j: 512 * (j + 1)],
                                pv2[1][j][0:DH, :],
                                rzb[:, 512 * j: 512 * (j + 1)], ALU.mult)
                        nc.gpsimd.dma_start(attnout[HH[1], hp, 1:NT],
                                            tmp[:])

                def epilogue():
                    """BOS-query output for all 16 heads, packed into one
                    PSUM bank (head h -> column h)."""
                    pvq = psA.tile([P, 512], F32, tag="ps")
                    for h in range(HEADS):
                        pq = pq_all[:, h, :]
                        nc.vector.tensor_tensor(
                            pq, pq,
                            rzqb_all[:, h: h + 1].to_broadcast([P, 9]),
                            ALU.mult)
                        nc.tensor.matmul(pvq[0:DH + 1, h: h + 1],
                                         vbos_panel[:, h, :],
                                         pq[0:1, 0:1], start=(h == 0),
                                         stop=False, skip_group_check=True)
                        for c in range(8):
                            nc.tensor.matmul(
                                pvq[0:DH + 1, h: h + 1], vh_panel[:, c, h, :],
                                pq[:, 1 + c: 2 + c], start=False,
                                stop=(h == HEADS - 1 and c == 7),
                                skip_group_check=True,
                            )
                    nc.vector.tensor_copy(attnout[0:DH, 0:8, 0:1],
                                          pvq[0:DH, 0:HEADS:2])
                    tmpb = small.tile([64, 8], BF16, tag="tmpb")
                    nc.vector.tensor_copy(tmpb[:], pvq[0:DH, 1:HEADS:2])
                    nc.gpsimd.dma_start(attnout[DH:P, 0:8, 0:1], tmpb[:])

                # ---- software-pipelined head-pair loop ----
                # normalize of hh1 is deferred into the NEXT head pair's
                # attention (after its first masks) so the DVE queue serves
                # the new pair's exp->mask->PV chain first
                qhT, khT, chunks = project_chunks(0)
                for c in chunks:
                    c()
                deferred = None
                for hp in range(8):
                    pv2 = attention(hp, qhT, khT, deferred)
                    if hp < 7:
                        qhT, khT, chunks = project_chunks(hp + 1)
                        for c in chunks:
                            c()
                        normalize_hh(hp, pv2, 0)
                        deferred = (lambda _hp, _pv2:
                                    (lambda: normalize_hh(_hp, _pv2, 1)))(hp, pv2)
                    else:
                        # last pair: odd head first (it has the extra
                        # partition-shift DMA hop the output projection's
                        # first close waits on)
                        epilogue()
                        normalize_hh(hp, pv2, 1)
                        normalize_hh(hp, pv2, 0)

                # prefetch the next batch's activations (the sync queue
                # serves these ahead of this batch's output DMAs)
                if b + 1 < BPC:
                    nqT = perbatch.tile([P, 8, NT], BF16, tag="qT", name="qT")
                    nkT = perbatch.tile([P, 8, NT], BF16, tag="kT", name="kT")
                    nvT = vpool.tile([P, 8, NT], BF16, tag="vT", name="vT")
                    load_batch(b + 1, nqT, nkT, nvT)
                    load_maps[b + 1] = (nqT, nkT, nvT)

                # ---- output projection ----
                # Every token chunk contracts over all head pairs, and hp7's
                # attnout lands last (normalize(7) + DMA). Split each chunk
                # into open (ct 0-6) / close (ct 7 + evacuation) with two
                # chunks in flight so ct-0..6 work covers that wait. Chunk 0
                # (BOS epilogue) and 8 (last token) are slotted late.
                def oproj_open(mt):
                    t0, tw = 128 * mt, 128 if mt < 8 else 1
                    accs = []
                    for half in range(2):
                        acc = psA.tile([P, 512], F32, tag="ps",
                                       name=f"oacc{half}")
                        for ct in range(7):
                            nc.tensor.matmul(
                                acc[0:tw, :],
                                attnout[:, ct, t0: t0 + tw],
                                wo_sb[:, ct, 512 * half: 512 * (half + 1)],
                                start=(ct == 0), stop=False,
                            )
                        accs.append(acc)
                    return accs

                def oproj_close(mt, accs):
                    t0, tw = 128 * mt, 128 if mt < 8 else 1
                    for half in range(2):
                        nc.tensor.matmul(
                            accs[half][0:tw, :],
                            attnout[:, 7, t0: t0 + tw],
                            wo_sb[:, 7, 512 * half: 512 * (half + 1)],
                            start=False, stop=True,
                        )
                        ost = bcast.tile([P, 512], F32, tag="ost")
                        if half == 0:
                            nc.scalar.copy(ost[0:tw, :], accs[half][0:tw, :])
                        else:
                            nc.vector.tensor_copy(ost[0:tw, :],
                                                  accs[half][0:tw, :])
                        nc.sync.dma_start(
                            out[b, t0: t0 + tw, 512 * half: 512 * (half + 1)],
                            ost[0:tw, :],
                        )

                pending = []
                for mt in (1, 2, 3, 0, 4, 5, 6, 7, 8):
                    pending.append((mt, oproj_open(mt)))
                    if len(pending) > 1:
                        oproj_close(*pending.pop(0))
                oproj_close(*pending.pop(0))

    nc.compile()
    return nc


_NC = None


def _get_nc():
    global _NC
    if _NC is None:
        _NC = build_nc()
    return _NC


def kernel(q, k, v, Wq, Wk, Wv, Wo, bo):
    bf16 = ml_dtypes.bfloat16
    qT = np.ascontiguousarray(np.asarray(q, np.float32).transpose(0, 2, 1)).astype(bf16)
    kT = np.ascontiguousarray(np.asarray(k, np.float32).transpose(0, 2, 1)).astype(bf16)
    vT = np.ascontiguousarray(np.asarray(v, np.float32).transpose(0, 2, 1)).astype(bf16)
    wq16 = np.asarray(Wq, np.float32).astype(bf16)
    wk16 = np.asarray(Wk, np.float32).astype(bf16)
    wv16 = np.asarray(Wv, np.float32).astype(bf16)
    wo16 = np.asarray(Wo, np.float32).astype(bf16)

    nc = _get_nc()
    in_maps = []
    for c in range(NCORES):
        sl = slice(BPC * c, BPC * (c + 1))
        in_maps.append({
            "qt": np.ascontiguousarray(qT[sl]),
            "kt": np.ascontiguousarray(kT[sl]),
            "vt": np.ascontiguousarray(vT[sl]),
            "wq": wq16, "wk": wk16, "wv": wv16, "wo": wo16,
        })
    res = run_bass_kernel_spmd(nc, in_maps, core_ids=list(range(NCORES)))
    out = np.concatenate([r["out"] for r in res.results], axis=0)
    out = out + np.asarray(bo, np.float32)[None, None, :]
    return out.astype(np.float32)


if __name__ == "__main__":
    rng = np.random.default_rng(0)
    ins = {
        "q": rng.standard_normal((B, NT, DIM), np.float32),
        "k": rng.standard_normal((B, NT, DIM), np.float32),
        "v": rng.standard_normal((B, NT, DIM), np.float32),
        "Wq": rng.standard_normal((DIM, INNER), np.float32) * DIM ** -0.5,
        "Wk": rng.standard_normal((DIM, INNER), np.float32) * DIM ** -0.5,
        "Wv": rng.standard_normal((DIM, INNER), np.float32) * DIM ** -0.5,
        "Wo": rng.standard_normal((INNER, DIM), np.float32) * INNER ** -0.5,
        "bo": np.zeros((DIM,), np.float32),
    }
    o = kernel(**ins)
    print(o.shape, o.dtype, np.abs(o).max())
